# revision 30
# baseline (speedup 1.0000x reference)
"""Trainium2 Bass kernel for prefix-causal self-attention (nn_CausalSelfAttention).

Reference semantics (B=4, T=2048, T_P=256, C=768, H=12, HD=64):
    x_full = concat([prefix, x], 1)                  (B, 2304, 768)
    qkv    = x_full @ W_qkv.T ; split q,k,v ; heads
    att    = softmax(mask(q k^T / sqrt(HD)))         prefix rows bidirectional,
                                                     x rows causal
    out    = (att v) heads-merged @ W_out.T ; return x-rows only (B, 2048, 768)

Sharding: 8 cores = 4 batches x 2 head-groups (tensor parallel on heads).
Each core computes Q/K/V for its 6 heads only (halving the K/V projection
work vs data-parallel-on-queries) over all 2304 kv rows and all 2048 query
rows, then the partial output projection y_g = O[:, g-heads] @ W_out^T[g].
The cross-group all-reduce of y is done on the HOST (numpy add of the two
partial results per batch) -- zero device cost. All 8 cores run an
identical instruction stream (true SPMD); only tensor data differs.

Query rows are processed in 16 chunks of 128 rows. With full-width rows per
chunk the causal-diagonal mask tile is the SAME lower-triangle [128x128] for
every chunk and head (one constant mask input), kv extents are exact
(et = 3+c tiles, no padding waste), and the S^T tile stream (6 heads x 168
tiles = 1008 tiles) packs perfectly into 126 PSUM quads of 8 tiles for exp.

On-chip pipeline per core (bf16 matmul operands, fp32 PSUM accumulation):
  QT/KT/V produced incrementally (one kv tile per chunk ahead of use) so
  attention starts early and PE gaps during exp are filled. Per S^T tile
  (kv 128 x q 128): S^T = K_h Q_h^T into a quad slot; one exp() per filled
  [128,1024] quad on ScalarE (softmax scale fused, no max-subtraction --
  scores are O(1) by construction); diagonal tiles get a triangle-mask
  multiply on DVE; AV emitted one quad behind the S stream with the
  orientation out[q,65] = pq^T V|1 (65-wide outputs halve the cost-model
  charge vs the [hd,q] orientation; the ones column gives the softmax
  denominator for free); per-head normalize via reciprocal + per-partition
  tensor_scalar broadcast (no DRAM bounce needed in this orientation);
  per-chunk O[q,384] is PE-transposed (identity matmul) to feed the W_out
  projection, y streamed out per chunk.
"""

import math
import os
from contextlib import ExitStack

import numpy as np
import ml_dtypes

import concourse.bass as bass
import concourse.bacc as bacc
import concourse.tile as tile
import concourse.mybir as mybir
from concourse._compat import with_exitstack

F32 = mybir.dt.float32
BF16 = mybir.dt.bfloat16
AF = mybir.ActivationFunctionType

# ---------------------------------------------------------------------------
# problem configuration
# ---------------------------------------------------------------------------


class Cfg:
    def __init__(self, B=4, T=2048, T_P=256, C=768, H=12):
        self.B, self.T, self.T_P, self.C, self.H = B, T, T_P, C, H
        self.HD = C // H
        assert self.HD == 64
        self.TALL = T_P + T
        assert self.TALL % 128 == 0 and T % 128 == 0 and T_P % 128 == 0
        self.NKV = self.TALL // 128          # kv tiles (18)
        self.CT = C // 128                   # contraction tiles over C (6)
        self.HG = H // 2                     # heads per core (6)
        self.CG = self.HG * self.HD          # feature cols per core (384)
        self.NP = self.CG // 128             # head pairs per core (3)
        self.NCH = T // 128                  # query chunks of 128 rows (16)
        self.PT = T_P // 128                 # prefix tiles (2)
        self.et = lambda c: self.PT + c + 1  # kv-tile extent of chunk c
        self.scale = 1.0 / math.sqrt(self.HD)


CFG = Cfg()

# ---------------------------------------------------------------------------
# device kernel (emitted once; same NEFF runs on all 8 cores)
# ---------------------------------------------------------------------------


@with_exitstack
def _emit(ctx: ExitStack, tc: tile.TileContext, cfg: Cfg, io: dict):
    nc = tc.nc
    C, CT, NP, NKV, NCH, CG = cfg.C, cfg.CT, cfg.NP, cfg.NKV, cfg.NCH, cfg.CG
    T = cfg.T

    xT_d, wq_d, wk_d, wv_d, wo_d, mk_d, y_d = (
        io["xT"], io["wqT"], io["wkT"], io["wvT"], io["woT"], io["mask"],
        io["y"])

    # ---- SBUF pools -------------------------------------------------------
    xT_p = ctx.enter_context(tc.tile_pool(name="xT", bufs=CT))
    w_p = ctx.enter_context(tc.tile_pool(name="w", bufs=3 * CT + NP))
    qT_p = ctx.enter_context(tc.tile_pool(name="qT", bufs=NP))
    kT_p = ctx.enter_context(tc.tile_pool(name="kT", bufs=NP))
    va_p = ctx.enter_context(tc.tile_pool(name="va", bufs=NKV))
    mk_p = ctx.enter_context(tc.tile_pool(name="mk", bufs=1))
    pq_p = ctx.enter_context(tc.tile_pool(name="pq", bufs=3))
    oc_p = ctx.enter_context(tc.tile_pool(name="oc", bufs=2))
    ot_p = ctx.enter_context(tc.tile_pool(name="ot", bufs=2 * NP))
    nrm_p = ctx.enter_context(tc.tile_pool(name="nrm", bufs=6))
    y_p = ctx.enter_context(tc.tile_pool(name="ysb", bufs=2))
    # PSUM pools: mm(2 banks) + quad(2x2 banks) + O(2x1 bank) = 8 banks
    mm_ps = ctx.enter_context(tc.tile_pool(name="mmps", bufs=2, space="PSUM"))
    qd_ps = ctx.enter_context(tc.tile_pool(name="qdps", bufs=2, space="PSUM"))
    o_ps = ctx.enter_context(tc.tile_pool(name="ops", bufs=2, space="PSUM"))

    # ---- input loads ------------------------------------------------------
    wq = [w_p.tile([128, CG], BF16, tag="w", name=f"wq{i}") for i in range(CT)]
    xT = [xT_p.tile([128, cfg.TALL], BF16, tag="xT", name=f"xT{i}")
          for i in range(CT)]
    wk = [w_p.tile([128, CG], BF16, tag="w", name=f"wk{i}") for i in range(CT)]
    wv = [w_p.tile([128, CG], BF16, tag="w", name=f"wv{i}") for i in range(CT)]
    wo = [w_p.tile([128, C], BF16, tag="w", name=f"wo{i}") for i in range(NP)]
    # xT arrives in column ranges so QT/KT/VA production can start early
    for ci in range(CT):
        nc.sync.dma_start(wq[ci][:], wq_d[bass.ts(ci, 128), :])
    for ci in range(CT):
        nc.sync.dma_start(wk[ci][:], wk_d[bass.ts(ci, 128), :])
    xr = [0, cfg.TALL // 384 * 128, 2 * (cfg.TALL // 384) * 128, cfg.TALL]
    for ci in range(CT):
        nc.sync.dma_start(xT[ci][:, xr[0]:xr[1]],
                          xT_d[bass.ts(ci, 128), xr[0]:xr[1]])
    for ci in range(CT):
        nc.sync.dma_start(wv[ci][:], wv_d[bass.ts(ci, 128), :])
    for ci in range(CT):
        nc.sync.dma_start(xT[ci][:, xr[1]:xr[2]],
                          xT_d[bass.ts(ci, 128), xr[1]:xr[2]])
    for p in range(NP):
        nc.sync.dma_start(wo[p][:], wo_d[bass.ts(p, 128), :])
    for ci in range(CT):
        nc.sync.dma_start(xT[ci][:, xr[2]:xr[3]],
                          xT_d[bass.ts(ci, 128), xr[2]:xr[3]])
    # mask input: [128, 128] bf16 lower-triangle + [128, 128] f32 identity
    mk = mk_p.tile([128, 128], BF16, name="mk")
    nc.sync.dma_start(mk[:], mk_d[:])
    tri = mk[:, 0:128]
    ident = mk_p.tile([128, 128], BF16, name="ident")
    nc.sync.dma_start(ident[:], io["identf"][:])

    # HAM warmup: burn the input-DMA wait on dependency-free matmuls so the
    # PE p-state ramp (full speed only after ~3us of sustained activity) is
    # over before real work starts.
    warm = mk_p.tile([128, 512], BF16, name="warm")
    nc.vector.memset(warm[:], 1.0)
    for i in range(9):
        wps = mm_ps.tile([128, 512], F32, tag="mm", name=f"warmps{i}")
        nc.tensor.matmul(wps[:], warm[:, 0:128], warm[:],
                         start=True, stop=True)

    # ---- incremental producers -------------------------------------------
    QT = [qT_p.tile([128, T], BF16, tag="qT", name=f"QT{p}")
          for p in range(NP)]
    KT = [kT_p.tile([128, cfg.TALL], BF16, tag="kT", name=f"KT{p}")
          for p in range(NP)]
    VA = [va_p.tile([128, cfg.HG * 65], BF16, tag="va", name=f"VA{m}")
          for m in range(NKV)]

    def qt_block(p, n, w):
        ps = mm_ps.tile([128, w], F32, tag="mm", name=f"qps{p}_{n}")
        for ci in range(CT):
            nc.tensor.matmul(
                ps[:], wq[ci][:, bass.ts(p, 128)],
                xT[ci][:, cfg.T_P + n: cfg.T_P + n + w],
                start=(ci == 0), stop=(ci == CT - 1))
        nc.vector.tensor_copy(QT[p][:, n:n + w], ps[:])

    def kt_tile(p, t):
        ps = mm_ps.tile([128, 128], F32, tag="mm", name=f"kps{p}_{t}")
        for ci in range(CT):
            nc.tensor.matmul(
                ps[:], wk[ci][:, bass.ts(p, 128)],
                xT[ci][:, bass.ts(t, 128)],
                start=(ci == 0), stop=(ci == CT - 1))
        nc.vector.tensor_copy(KT[p][:, bass.ts(t, 128)], ps[:])

    def va_tile(m, half):
        hw = CG // 2                       # 3 heads x 64 psum cols
        hv = (cfg.HG // 2) * 65            # 3 heads x 65 VA cols
        vview = VA[m][:, half * hv:(half + 1) * hv].rearrange(
            "p (h c) -> p h c", c=65)
        nc.vector.memset(vview[:, :, 64:65], 1.0)
        ps = mm_ps.tile([128, hw], F32, tag="mm", name=f"vps{m}_{half}")
        for ci in range(CT):
            nc.tensor.matmul(
                ps[:], xT[ci][:, bass.ts(m, 128)],
                wv[ci][:, half * hw:(half + 1) * hw],
                start=(ci == 0), stop=(ci == CT - 1))
        nc.vector.tensor_copy(
            vview[:, :, 0:64], ps[:].rearrange("p (h c) -> p h c", c=64))

    # Work queue of PE filler items (KT/VA/QT production, deferred
    # projections): each item = (deadline_chunk_or_None, units, fn).
    # Deadline items MUST be emitted before that chunk's S matmuls enter the
    # PE queue (in-order engine); non-deadline items are spread evenly
    # across exp-paced quads to keep the PE fed.
    work = []
    wstat = {"emitted": 0, "quads": 0, "cycle": 0}
    # production pieces in deadline order (deadline = first chunk needing
    # the piece); pieces with deadline <= 0 are emitted before the stream.
    pieces = []
    for t in range(NKV):
        dl = t - cfg.PT
        for p in range(NP):
            pieces.append((dl, CT * 128, lambda p=p, t=t: kt_tile(p, t)))
        for half in range(2):
            pieces.append((dl, CT * CG // 2,
                           lambda t=t, h=half: va_tile(t, h)))
    for n in range(0, T, 256):
        for p in range(NP):
            pieces.append((n // 128, CT * 256,
                           lambda p=p, n=n: qt_block(p, n, 256)))
    pieces.sort(key=lambda it: it[0])
    for dl, u, fn in pieces:
        if dl <= 0:
            fn()
        else:
            work.append((dl, u, fn))

    # ---- attention chunk stream ------------------------------------------
    # Global pair-step stream packed into [128,1024] PSUM quads: each quad
    # holds 4 pair-steps; a pair-step emits the even head's S^T tile into a
    # bank-A slot (tile_position row 0) and the odd head's into the matching
    # bank-B slot (row 64). HW constraint (found the hard way): one PSUM
    # bank must not receive matmuls with different tile_position row bases
    # -- mixing 0/64 within a bank wedges the device. 504 pair-steps pack
    # exactly into 126 quads.
    total_steps = NP * sum(cfg.et(c) for c in range(NCH))
    state = {"qd": None, "pq": None, "cur": 0, "pending": [], "prev": [],
             "masks": [], "step": 0}

    def flush_quad():
        """Close the current quad: exp + masks; drain previous quad's AV."""
        qd, pq = state["qd"], state["pq"]
        if qd is None:
            return
        if state["cur"] == 4:
            nc.scalar.activation(pq[:], qd[:], AF.Exp, scale=cfg.scale)
        else:
            w = 128 * state["cur"]
            qv = qd[:].rearrange("p (b s) -> p b s", s=512)[:, :, 0:w]
            pv = pq[:].rearrange("p (b s) -> p b s", s=512)[:, :, 0:w]
            nc.scalar.activation(pv, qv, AF.Exp, scale=cfg.scale)
        for emit_mask in state["masks"]:
            emit_mask()
        # drain the PREVIOUS quad's AV/finish items now that this quad's S
        # matmuls are queued ahead of them on PE (keeps PE fed during exp).
        for it in state["prev"]:
            it()
        state["prev"] = state["pending"]
        state["pending"] = []
        state["qd"] = None
        state["masks"] = []
        wstat["quads"] += 1
        wstat["cycle"] = 0
        drain_work(budget=CYCLE_BUDGET)

    CYCLE_BUDGET = 1300   # PE units of filler per exp-paced quad cycle

    def drain_work(deadline=None, budget=0):
        while work:
            dl, u, fn = work[0]
            due = deadline is not None and dl is not None and dl <= deadline
            if not due and budget > 0:
                due = wstat["cycle"] < budget
            if not due:
                break
            work.pop(0)
            fn()
            wstat["cycle"] += u
            wstat["emitted"] += u

    for c in range(NCH):
        et = cfg.et(c)
        qcols = bass.ts(c, 128)
        OC = oc_p.tile([128, CG], BF16, tag="oc", name=f"OC{c}")
        OTs = [ot_p.tile([128, 128], BF16, tag="ot", name=f"OT{c}_{p}")
               for p in range(NP)]
        for p in range(NP):
            O = o_ps.tile([128, 130], F32, tag="O", name=f"O{c}_{p}")
            for k in range(et):
                if state["qd"] is None:
                    state["qd"] = qd_ps.tile([128, 1024], F32, tag="qd",
                                             name="qd")
                    state["pq"] = pq_p.tile([128, 1024], BF16, tag="pq",
                                            name="pq")
                    state["cur"] = 0
                cur = state["cur"]
                qd, pq = state["qd"], state["pq"]
                for ho in range(2):
                    h, hp, s = 2 * p + ho, 64 * ho, cur + 4 * ho
                    nc.tensor.matmul(
                        qd[:, bass.ts(s, 128)],
                        KT[p][hp:hp + 64, bass.ts(k, 128)],
                        QT[p][hp:hp + 64, qcols],
                        start=(cur == 0),
                        stop=(cur == 3 or state["step"] == total_steps - 1))
                    if k == et - 1:
                        state["masks"].append(
                            lambda pq=pq, s=s:
                            nc.vector.tensor_mul(pq[:, bass.ts(s, 128)],
                                                 pq[:, bass.ts(s, 128)], tri))

                def av_item(pq=pq, cur=cur, O=O, p=p, k=k, et=et):
                    for ho in range(2):
                        nc.tensor.matmul(
                            O[:, 65 * ho:65 * ho + 65],
                            pq[:, bass.ts(cur + 4 * ho, 128)],
                            VA[k][:, 65 * (2 * p + ho):65 * (2 * p + ho) + 65],
                            start=(k == 0 and ho == 0),
                            stop=(k == et - 1 and ho == 1))
                state["pending"].append(av_item)
                state["cur"] += 1
                state["step"] += 1
                if state["cur"] == 4:
                    flush_quad()

            def fin_pair(O=O, p=p, c=c, OC=OC):
                recip = nrm_p.tile([128, 2], F32, tag="recip",
                                   name=f"rc{c}_{p}")
                dview = O[:].rearrange("q (h c) -> q h c", c=65)
                nc.vector.reciprocal(recip[:], dview[:, :, 64])
                for ho in range(2):
                    nc.vector.tensor_scalar_mul(
                        OC[:, 128 * p + 64 * ho:128 * p + 64 * ho + 64],
                        O[:, 65 * ho:65 * ho + 64], recip[:, ho:ho + 1])
            state["pending"].append(fin_pair)

        def transp(c=c, OC=OC, OTs=OTs):
            for p in range(NP):
                tp = mm_ps.tile([128, 128], BF16, tag="mm", name=f"tp{c}_{p}")
                nc.tensor.transpose(tp[:], OC[:, bass.ts(p, 128)], ident)
                nc.vector.tensor_copy(OTs[p][:], tp[:])

        def mk_proj(c, OTs, n, w, ysb):
            def proj():
                ps = mm_ps.tile([128, w], F32, tag="mm", name=f"yps{c}_{n}")
                for p in range(NP):
                    nc.tensor.matmul(ps[:], OTs[p][:], wo[p][:, n:n + w],
                                     start=(p == 0), stop=(p == NP - 1))
                nc.vector.tensor_copy(ysb[:, n:n + w], ps[:])
                if n + w == C:
                    nc.sync.dma_start(y_d[bass.ts(c, 128), :], ysb[:])
            return proj

        def fin_chunk(c=c, OC=OC, OTs=OTs):
            # deferred PE filler: transposes + output projection; emitted
            # later by the budget drain so late (exp-paced) quads stay fed
            ysb = y_p.tile([128, C], F32, tag="ysb", name=f"ysb{c}")
            work.append((None, NP * 128, lambda: transp(c, OC, OTs)))
            for n in range(0, C, 256):
                work.append((None, NP * 256, mk_proj(c, OTs, n, 256, ysb)))
        state["pending"].append(fin_chunk)
        drain_work(deadline=c + 1)

    flush_quad()  # close the final quad (no-op when it ended exactly full)
    for it in state["prev"]:
        it()
    for it in state["pending"]:
        it()
    state["pending"] = []
    while work:
        _, u, fn = work.pop(0)
        fn()
    assert state["qd"] is None


def build_nc(cfg: Cfg):
    nc = bacc.Bacc("TRN2", target_bir_lowering=False, debug=False,
                   enable_asserts=False)
    io = {
        "xT": nc.dram_tensor("xT", (cfg.C, cfg.TALL), BF16,
                             kind="ExternalInput").ap(),
        "wqT": nc.dram_tensor("wqT", (cfg.C, cfg.CG), BF16,
                              kind="ExternalInput").ap(),
        "wkT": nc.dram_tensor("wkT", (cfg.C, cfg.CG), BF16,
                              kind="ExternalInput").ap(),
        "wvT": nc.dram_tensor("wvT", (cfg.C, cfg.CG), BF16,
                              kind="ExternalInput").ap(),
        "woT": nc.dram_tensor("woT", (cfg.CG, cfg.C), BF16,
                              kind="ExternalInput").ap(),
        "mask": nc.dram_tensor("mask", (128, 128), BF16,
                               kind="ExternalInput").ap(),
        "identf": nc.dram_tensor("identf", (128, 128), BF16,
                                 kind="ExternalInput").ap(),
        "y": nc.dram_tensor("y", (cfg.T, cfg.C), F32,
                            kind="ExternalOutput").ap(),
    }
    with tile.TileContext(nc) as tc:
        _emit(tc, cfg, io)
    nc.compile()
    return nc


# ---------------------------------------------------------------------------
# host side: shard, run, gather
# ---------------------------------------------------------------------------


def _in_maps(cfg: Cfg, x, prefix, W_qkv, W_out):
    C, CG = cfg.C, cfg.CG
    mask = np.triu(np.ones((128, 128), np.float32)  # mask[i,j]=1 iff j>=i
                   ).astype(ml_dtypes.bfloat16)
    identf = np.eye(128, dtype=np.float32).astype(ml_dtypes.bfloat16)
    xTs = []
    for b in range(cfg.B):
        xT = np.ascontiguousarray(
            np.concatenate([prefix[b], x[b]], axis=0).T
        ).astype(ml_dtypes.bfloat16)
        xTs.append(xT)
    maps = []
    for core in range(2 * cfg.B):
        b, g = divmod(core, 2)
        sl = slice(CG * g, CG * (g + 1))
        maps.append({
            "xT": xTs[b],
            "wqT": np.ascontiguousarray(W_qkv[0:C][sl].T
                                        ).astype(ml_dtypes.bfloat16),
            "wkT": np.ascontiguousarray(W_qkv[C:2 * C][sl].T
                                        ).astype(ml_dtypes.bfloat16),
            "wvT": np.ascontiguousarray(W_qkv[2 * C:][sl].T
                                        ).astype(ml_dtypes.bfloat16),
            "woT": np.ascontiguousarray(W_out[:, sl].T
                                        ).astype(ml_dtypes.bfloat16),
            "mask": mask,
            "identf": identf,
        })
    return maps


_NC_CACHE = {}


def run(cfg: Cfg, x, prefix, W_qkv, W_out, **kw):
    from concourse.bass_utils import run_bass_kernel_spmd
    key = (cfg.B, cfg.T, cfg.T_P, cfg.C, cfg.H)
    if key not in _NC_CACHE:
        _NC_CACHE[key] = build_nc(cfg)
    nc = _NC_CACHE[key]
    maps = _in_maps(cfg, x, prefix, W_qkv, W_out)
    res = run_bass_kernel_spmd(nc, maps, core_ids=list(range(2 * cfg.B)), **kw)
    out = np.empty((cfg.B, cfg.T, cfg.C), np.float32)
    for b in range(cfg.B):
        out[b] = res.results[2 * b]["y"] + res.results[2 * b + 1]["y"]
    return out, res


def kernel(x, prefix, W_qkv, W_out):
    x = np.asarray(x, np.float32)
    prefix = np.asarray(prefix, np.float32)
    W_qkv = np.asarray(W_qkv, np.float32)
    W_out = np.asarray(W_out, np.float32)
    out, _ = run(CFG, x, prefix, W_qkv, W_out)
    return out


# revision 33
# speedup vs baseline: 1.0568x; 1.0568x over previous
"""Trainium2 Bass kernel for prefix-causal self-attention (nn_CausalSelfAttention).

Reference semantics (B=4, T=2048, T_P=256, C=768, H=12, HD=64):
    x_full = concat([prefix, x], 1)                  (B, 2304, 768)
    qkv    = x_full @ W_qkv.T ; split q,k,v ; heads
    att    = softmax(mask(q k^T / sqrt(HD)))         prefix rows bidirectional,
                                                     x rows causal
    out    = (att v) heads-merged @ W_out.T ; return x-rows only (B, 2048, 768)

Sharding: 8 cores = 4 batches x 2 head-groups (tensor parallel on heads).
Each core computes Q/K/V for its 6 heads only (halving the K/V projection
work vs data-parallel-on-queries) over all 2304 kv rows and all 2048 query
rows, then the partial output projection y_g = O[:, g-heads] @ W_out^T[g].
The cross-group all-reduce of y is done on the HOST (numpy add of the two
partial results per batch) -- zero device cost. All 8 cores run an
identical instruction stream (true SPMD); only tensor data differs.

Query rows are processed in 16 chunks of 128 rows. With full-width rows per
chunk the causal-diagonal mask tile is the SAME lower-triangle [128x128] for
every chunk and head (one constant mask input), kv extents are exact
(et = 3+c tiles, no padding waste), and the S^T tile stream (6 heads x 168
tiles = 1008 tiles) packs perfectly into 126 PSUM quads of 8 tiles for exp.

On-chip pipeline per core (bf16 matmul operands, fp32 PSUM accumulation):
  QT/KT/V produced incrementally (one kv tile per chunk ahead of use) so
  attention starts early and PE gaps during exp are filled. Per S^T tile
  (kv 128 x q 128): S^T = K_h Q_h^T into a quad slot; one exp() per filled
  [128,1024] quad on ScalarE (softmax scale fused, no max-subtraction --
  scores are O(1) by construction); diagonal tiles get a triangle-mask
  multiply on DVE; AV emitted one quad behind the S stream with the
  orientation out[q,65] = pq^T V|1 (65-wide outputs halve the cost-model
  charge vs the [hd,q] orientation; the ones column gives the softmax
  denominator for free); per-head normalize via reciprocal + per-partition
  tensor_scalar broadcast (no DRAM bounce needed in this orientation);
  per-chunk O[q,384] is PE-transposed (identity matmul) to feed the W_out
  projection, y streamed out per chunk.
"""

import math
import os
from contextlib import ExitStack

import numpy as np
import ml_dtypes

import concourse.bass as bass
import concourse.bacc as bacc
import concourse.tile as tile
import concourse.mybir as mybir
from concourse._compat import with_exitstack

F32 = mybir.dt.float32
BF16 = mybir.dt.bfloat16
AF = mybir.ActivationFunctionType

# ---------------------------------------------------------------------------
# problem configuration
# ---------------------------------------------------------------------------


class Cfg:
    def __init__(self, B=4, T=2048, T_P=256, C=768, H=12):
        self.B, self.T, self.T_P, self.C, self.H = B, T, T_P, C, H
        self.HD = C // H
        assert self.HD == 64
        self.TALL = T_P + T
        assert self.TALL % 128 == 0 and T % 128 == 0 and T_P % 128 == 0
        self.NKV = self.TALL // 128          # kv tiles (18)
        self.CT = C // 128                   # contraction tiles over C (6)
        self.HG = H // 2                     # heads per core (6)
        self.CG = self.HG * self.HD          # feature cols per core (384)
        self.NP = self.CG // 128             # head pairs per core (3)
        self.NCH = T // 128                  # query chunks of 128 rows (16)
        self.PT = T_P // 128                 # prefix tiles (2)
        self.et = lambda c: self.PT + c + 1  # kv-tile extent of chunk c
        self.scale = 1.0 / math.sqrt(self.HD)


CFG = Cfg()

# ---------------------------------------------------------------------------
# device kernel (emitted once; same NEFF runs on all 8 cores)
# ---------------------------------------------------------------------------


@with_exitstack
def _emit(ctx: ExitStack, tc: tile.TileContext, cfg: Cfg, io: dict):
    nc = tc.nc
    C, CT, NP, NKV, NCH, CG = cfg.C, cfg.CT, cfg.NP, cfg.NKV, cfg.NCH, cfg.CG
    T = cfg.T

    xT_d, wq_d, wk_d, wv_d, wo_d, mk_d, y_d = (
        io["xT"], io["wqT"], io["wkT"], io["wvT"], io["woT"], io["mask"],
        io["y"])

    # ---- SBUF pools -------------------------------------------------------
    xT_p = ctx.enter_context(tc.tile_pool(name="xT", bufs=1))
    w_p = ctx.enter_context(tc.tile_pool(name="w", bufs=1))
    qT_p = ctx.enter_context(tc.tile_pool(name="qT", bufs=NP))
    kT_p = ctx.enter_context(tc.tile_pool(name="kT", bufs=NP))
    va_p = ctx.enter_context(tc.tile_pool(name="va", bufs=NKV))
    mk_p = ctx.enter_context(tc.tile_pool(name="mk", bufs=1))
    pq_p = ctx.enter_context(tc.tile_pool(name="pq", bufs=4))
    oc_p = ctx.enter_context(tc.tile_pool(name="oc", bufs=4))
    ot_p = ctx.enter_context(tc.tile_pool(name="ot", bufs=4 * NP))
    nrm_p = ctx.enter_context(tc.tile_pool(name="nrm", bufs=8))
    y_p = ctx.enter_context(tc.tile_pool(name="ysb", bufs=4))
    # PSUM pools: mm(2 banks) + quad(2x2 banks) + O(2x1 bank) = 8 banks
    mm_ps = ctx.enter_context(tc.tile_pool(name="mmps", bufs=2, space="PSUM"))
    qd_ps = ctx.enter_context(tc.tile_pool(name="qdps", bufs=2, space="PSUM"))
    o_ps = ctx.enter_context(tc.tile_pool(name="ops", bufs=2, space="PSUM"))

    # ---- input loads ------------------------------------------------------
    # each logical tensor is ONE wide SBUF tile (per-ci column views) so it
    # can be filled by a single batched 3D-AP DMA
    wq_all = w_p.tile([128, CT * CG], BF16, tag="wq", name="wq_all")
    xT_all = xT_p.tile([128, CT * cfg.TALL], BF16, tag="xT", name="xT_all")
    wk_all = w_p.tile([128, CT * CG], BF16, tag="wk", name="wk_all")
    wv_all = w_p.tile([128, CT * CG], BF16, tag="wv", name="wv_all")
    wo_all = w_p.tile([128, NP * C], BF16, tag="wo", name="wo_all")
    wq = [wq_all[:, i * CG:(i + 1) * CG] for i in range(CT)]
    xT = [xT_all[:, i * cfg.TALL:(i + 1) * cfg.TALL] for i in range(CT)]
    wk = [wk_all[:, i * CG:(i + 1) * CG] for i in range(CT)]
    wv = [wv_all[:, i * CG:(i + 1) * CG] for i in range(CT)]
    wo = [wo_all[:, i * C:(i + 1) * C] for i in range(NP)]
    # xT arrives in column ranges so QT/KT/VA production can start early
    for ci in range(CT):
        nc.sync.dma_start(wq[ci][:], wq_d[bass.ts(ci, 128), :])
    for ci in range(CT):
        nc.sync.dma_start(wk[ci][:], wk_d[bass.ts(ci, 128), :])
    xr = [0, cfg.TALL // 384 * 128, 2 * (cfg.TALL // 384) * 128, cfg.TALL]
    for ci in range(CT):
        nc.sync.dma_start(xT[ci][:, xr[0]:xr[1]],
                          xT_d[bass.ts(ci, 128), xr[0]:xr[1]])
    for ci in range(CT):
        nc.sync.dma_start(wv[ci][:], wv_d[bass.ts(ci, 128), :])
    for ci in range(CT):
        nc.sync.dma_start(xT[ci][:, xr[1]:xr[2]],
                          xT_d[bass.ts(ci, 128), xr[1]:xr[2]])
    for p in range(NP):
        nc.sync.dma_start(wo[p][:], wo_d[bass.ts(p, 128), :])
    for ci in range(CT):
        nc.sync.dma_start(xT[ci][:, xr[2]:xr[3]],
                          xT_d[bass.ts(ci, 128), xr[2]:xr[3]])
    # mask input: [128, 128] bf16 lower-triangle + [128, 128] f32 identity
    mk = mk_p.tile([128, 128], BF16, name="mk")
    nc.sync.dma_start(mk[:], mk_d[:])
    tri = mk[:, 0:128]
    ident = mk_p.tile([128, 128], BF16, name="ident")
    nc.sync.dma_start(ident[:], io["identf"][:])

    # HAM warmup: burn the input-DMA wait on dependency-free matmuls so the
    # PE p-state ramp (full speed only after ~3us of sustained activity) is
    # over before real work starts.
    warm = mk_p.tile([128, 512], BF16, name="warm")
    nc.vector.memset(warm[:], 1.0)
    for i in range(9):
        wps = mm_ps.tile([128, 512], F32, tag="mm", name=f"warmps{i}")
        nc.tensor.matmul(wps[:], warm[:, 0:128], warm[:],
                         start=True, stop=True)

    # ---- incremental producers -------------------------------------------
    QT = [qT_p.tile([128, T], BF16, tag="qT", name=f"QT{p}")
          for p in range(NP)]
    KT = [kT_p.tile([128, cfg.TALL], BF16, tag="kT", name=f"KT{p}")
          for p in range(NP)]
    VA = [va_p.tile([128, cfg.HG * 65], BF16, tag="va", name=f"VA{m}")
          for m in range(NKV)]

    def qt_block(p, n, w):
        ps = mm_ps.tile([128, w], F32, tag="mm", name=f"qps{p}_{n}")
        for ci in range(CT):
            nc.tensor.matmul(
                ps[:], wq[ci][:, bass.ts(p, 128)],
                xT[ci][:, cfg.T_P + n: cfg.T_P + n + w],
                start=(ci == 0), stop=(ci == CT - 1))
        nc.vector.tensor_copy(QT[p][:, n:n + w], ps[:])

    def kt_tile(p, t):
        ps = mm_ps.tile([128, 128], F32, tag="mm", name=f"kps{p}_{t}")
        for ci in range(CT):
            nc.tensor.matmul(
                ps[:], wk[ci][:, bass.ts(p, 128)],
                xT[ci][:, bass.ts(t, 128)],
                start=(ci == 0), stop=(ci == CT - 1))
        nc.vector.tensor_copy(KT[p][:, bass.ts(t, 128)], ps[:])

    def va_tile(m, half):
        hw = CG // 2                       # 3 heads x 64 psum cols
        hv = (cfg.HG // 2) * 65            # 3 heads x 65 VA cols
        vview = VA[m][:, half * hv:(half + 1) * hv].rearrange(
            "p (h c) -> p h c", c=65)
        nc.vector.memset(vview[:, :, 64:65], 1.0)
        ps = mm_ps.tile([128, hw], F32, tag="mm", name=f"vps{m}_{half}")
        for ci in range(CT):
            nc.tensor.matmul(
                ps[:], xT[ci][:, bass.ts(m, 128)],
                wv[ci][:, half * hw:(half + 1) * hw],
                start=(ci == 0), stop=(ci == CT - 1))
        nc.vector.tensor_copy(
            vview[:, :, 0:64], ps[:].rearrange("p (h c) -> p h c", c=64))

    # Work queue of PE filler items (KT/VA/QT production, deferred
    # projections): each item = (deadline_chunk_or_None, units, fn).
    # Deadline items MUST be emitted before that chunk's S matmuls enter the
    # PE queue (in-order engine); non-deadline items are spread evenly
    # across exp-paced quads to keep the PE fed.
    work = []
    wstat = {"emitted": 0, "quads": 0, "cycle": 0}
    # production pieces in deadline order (deadline = first chunk needing
    # the piece); pieces with deadline <= 0 are emitted before the stream.
    pieces = []
    for t in range(NKV):
        dl = t - cfg.PT
        for p in range(NP):
            pieces.append((dl, CT * 128, lambda p=p, t=t: kt_tile(p, t)))
        for half in range(2):
            pieces.append((dl, CT * CG // 2,
                           lambda t=t, h=half: va_tile(t, h)))
    for n in range(0, T, 256):
        for p in range(NP):
            pieces.append((n // 128, CT * 256,
                           lambda p=p, n=n: qt_block(p, n, 256)))
    pieces.sort(key=lambda it: it[0])
    for dl, u, fn in pieces:
        if dl <= 0:
            fn()
        else:
            work.append((dl, u, fn))

    # ---- attention chunk stream ------------------------------------------
    # Global pair-step stream packed into [128,1024] PSUM quads: each quad
    # holds 4 pair-steps; a pair-step emits the even head's S^T tile into a
    # bank-A slot (tile_position row 0) and the odd head's into the matching
    # bank-B slot (row 64). HW constraint (found the hard way): one PSUM
    # bank must not receive matmuls with different tile_position row bases
    # -- mixing 0/64 within a bank wedges the device. 504 pair-steps pack
    # exactly into 126 quads.
    total_steps = NP * sum(cfg.et(c) for c in range(NCH))
    state = {"qd": None, "pq": None, "cur": 0, "pending": [], "prev": [],
             "prev2": [], "masks": [], "step": 0}

    def flush_quad():
        """Close the current quad: exp + masks; drain previous quad's AV."""
        qd, pq = state["qd"], state["pq"]
        if qd is None:
            return
        if state["cur"] == 4:
            nc.scalar.activation(pq[:], qd[:], AF.Exp, scale=cfg.scale)
        else:
            w = 128 * state["cur"]
            qv = qd[:].rearrange("p (b s) -> p b s", s=512)[:, :, 0:w]
            pv = pq[:].rearrange("p (b s) -> p b s", s=512)[:, :, 0:w]
            nc.scalar.activation(pv, qv, AF.Exp, scale=cfg.scale)
        for emit_mask in state["masks"]:
            emit_mask()
        # drain the quad-before-last's AV/finish items: two quads of S
        # matmuls are queued ahead of each AV on PE, so exp has ~2 cycles
        # of lead time and the PE never parks on the Activation sem.
        for it in state["prev2"]:
            it()
        state["prev2"] = state["prev"]
        state["prev"] = state["pending"]
        state["pending"] = []
        state["qd"] = None
        state["masks"] = []
        wstat["quads"] += 1
        wstat["cycle"] = 0
        drain_work(budget=CYCLE_BUDGET)

    CYCLE_BUDGET = 1300   # PE units of filler per exp-paced quad cycle

    def drain_work(deadline=None, budget=0):
        while work:
            dl, u, fn = work[0]
            due = deadline is not None and dl is not None and dl <= deadline
            if not due and budget > 0:
                due = wstat["cycle"] < budget
            if not due:
                break
            work.pop(0)
            fn()
            wstat["cycle"] += u
            wstat["emitted"] += u

    for c in range(NCH):
        et = cfg.et(c)
        qcols = bass.ts(c, 128)
        OC = oc_p.tile([128, CG], BF16, tag="oc", name=f"OC{c}")
        OTs = [ot_p.tile([128, 128], BF16, tag="ot", name=f"OT{c}_{p}")
               for p in range(NP)]
        for p in range(NP):
            O = o_ps.tile([128, 130], F32, tag="O", name=f"O{c}_{p}")
            for k in range(et):
                if state["qd"] is None:
                    state["qd"] = qd_ps.tile([128, 1024], F32, tag="qd",
                                             name="qd")
                    state["pq"] = pq_p.tile([128, 1024], BF16, tag="pq",
                                            name="pq")
                    state["cur"] = 0
                cur = state["cur"]
                qd, pq = state["qd"], state["pq"]
                for ho in range(2):
                    h, hp, s = 2 * p + ho, 64 * ho, cur + 4 * ho
                    nc.tensor.matmul(
                        qd[:, bass.ts(s, 128)],
                        KT[p][hp:hp + 64, bass.ts(k, 128)],
                        QT[p][hp:hp + 64, qcols],
                        start=(cur == 0),
                        stop=(cur == 3 or state["step"] == total_steps - 1))
                    if k == et - 1:
                        state["masks"].append(
                            lambda pq=pq, s=s:
                            nc.vector.tensor_mul(pq[:, bass.ts(s, 128)],
                                                 pq[:, bass.ts(s, 128)], tri))

                def av_item(pq=pq, cur=cur, O=O, p=p, k=k, et=et):
                    for ho in range(2):
                        nc.tensor.matmul(
                            O[:, 65 * ho:65 * ho + 65],
                            pq[:, bass.ts(cur + 4 * ho, 128)],
                            VA[k][:, 65 * (2 * p + ho):65 * (2 * p + ho) + 65],
                            start=(k == 0 and ho == 0),
                            stop=(k == et - 1 and ho == 1))
                state["pending"].append(av_item)
                state["cur"] += 1
                state["step"] += 1
                if state["cur"] == 4:
                    flush_quad()

            def fin_pair(O=O, p=p, c=c, OC=OC):
                recip = nrm_p.tile([128, 2], F32, tag="recip",
                                   name=f"rc{c}_{p}")
                dview = O[:].rearrange("q (h c) -> q h c", c=65)
                nc.vector.reciprocal(recip[:], dview[:, :, 64])
                for ho in range(2):
                    nc.vector.tensor_scalar_mul(
                        OC[:, 128 * p + 64 * ho:128 * p + 64 * ho + 64],
                        O[:, 65 * ho:65 * ho + 64], recip[:, ho:ho + 1])
            state["pending"].append(fin_pair)

        def transp(c=c, OC=OC, OTs=OTs):
            for p in range(NP):
                tp = mm_ps.tile([128, 128], BF16, tag="mm", name=f"tp{c}_{p}")
                nc.tensor.transpose(tp[:], OC[:, bass.ts(p, 128)], ident)
                nc.vector.tensor_copy(OTs[p][:], tp[:])

        def mk_proj(c, OTs, n, w, ysb):
            def proj():
                ps = mm_ps.tile([128, w], F32, tag="mm", name=f"yps{c}_{n}")
                for p in range(NP):
                    nc.tensor.matmul(ps[:], OTs[p][:], wo[p][:, n:n + w],
                                     start=(p == 0), stop=(p == NP - 1))
                nc.vector.tensor_copy(ysb[:, n:n + w], ps[:])
                if n + w == C:
                    nc.sync.dma_start(y_d[bass.ts(c, 128), :], ysb[:])
            return proj

        def fin_chunk(c=c, OC=OC, OTs=OTs):
            # deferred PE filler: transposes + output projection; emitted
            # later by the budget drain so late (exp-paced) quads stay fed
            ysb = y_p.tile([128, C], F32, tag="ysb", name=f"ysb{c}")
            work.append((None, NP * 128, lambda: transp(c, OC, OTs)))
            for n in range(0, C, 256):
                work.append((None, NP * 256, mk_proj(c, OTs, n, 256, ysb)))
        state["pending"].append(fin_chunk)
        drain_work(deadline=c + 1)

    flush_quad()  # close the final quad (no-op when it ended exactly full)
    for it in state["prev2"]:
        it()
    for it in state["prev"]:
        it()
    for it in state["pending"]:
        it()
    state["pending"] = []
    while work:
        _, u, fn = work.pop(0)
        fn()
    assert state["qd"] is None


def build_nc(cfg: Cfg):
    nc = bacc.Bacc("TRN2", target_bir_lowering=False, debug=False,
                   enable_asserts=False)
    io = {
        "xT": nc.dram_tensor("xT", (cfg.C, cfg.TALL), BF16,
                             kind="ExternalInput").ap(),
        "wqT": nc.dram_tensor("wqT", (cfg.C, cfg.CG), BF16,
                              kind="ExternalInput").ap(),
        "wkT": nc.dram_tensor("wkT", (cfg.C, cfg.CG), BF16,
                              kind="ExternalInput").ap(),
        "wvT": nc.dram_tensor("wvT", (cfg.C, cfg.CG), BF16,
                              kind="ExternalInput").ap(),
        "woT": nc.dram_tensor("woT", (cfg.CG, cfg.C), BF16,
                              kind="ExternalInput").ap(),
        "mask": nc.dram_tensor("mask", (128, 128), BF16,
                               kind="ExternalInput").ap(),
        "identf": nc.dram_tensor("identf", (128, 128), BF16,
                                 kind="ExternalInput").ap(),
        "y": nc.dram_tensor("y", (cfg.T, cfg.C), F32,
                            kind="ExternalOutput").ap(),
    }
    with tile.TileContext(nc) as tc:
        _emit(tc, cfg, io)
    nc.compile()
    return nc


# ---------------------------------------------------------------------------
# host side: shard, run, gather
# ---------------------------------------------------------------------------


def _in_maps(cfg: Cfg, x, prefix, W_qkv, W_out):
    C, CG = cfg.C, cfg.CG
    mask = np.triu(np.ones((128, 128), np.float32)  # mask[i,j]=1 iff j>=i
                   ).astype(ml_dtypes.bfloat16)
    identf = np.eye(128, dtype=np.float32).astype(ml_dtypes.bfloat16)
    xTs = []
    for b in range(cfg.B):
        xT = np.ascontiguousarray(
            np.concatenate([prefix[b], x[b]], axis=0).T
        ).astype(ml_dtypes.bfloat16)
        xTs.append(xT)
    maps = []
    for core in range(2 * cfg.B):
        b, g = divmod(core, 2)
        sl = slice(CG * g, CG * (g + 1))
        maps.append({
            "xT": xTs[b],
            "wqT": np.ascontiguousarray(W_qkv[0:C][sl].T
                                        ).astype(ml_dtypes.bfloat16),
            "wkT": np.ascontiguousarray(W_qkv[C:2 * C][sl].T
                                        ).astype(ml_dtypes.bfloat16),
            "wvT": np.ascontiguousarray(W_qkv[2 * C:][sl].T
                                        ).astype(ml_dtypes.bfloat16),
            "woT": np.ascontiguousarray(W_out[:, sl].T
                                        ).astype(ml_dtypes.bfloat16),
            "mask": mask,
            "identf": identf,
        })
    return maps


_NC_CACHE = {}


def run(cfg: Cfg, x, prefix, W_qkv, W_out, **kw):
    from concourse.bass_utils import run_bass_kernel_spmd
    key = (cfg.B, cfg.T, cfg.T_P, cfg.C, cfg.H)
    if key not in _NC_CACHE:
        _NC_CACHE[key] = build_nc(cfg)
    nc = _NC_CACHE[key]
    maps = _in_maps(cfg, x, prefix, W_qkv, W_out)
    res = run_bass_kernel_spmd(nc, maps, core_ids=list(range(2 * cfg.B)), **kw)
    out = np.empty((cfg.B, cfg.T, cfg.C), np.float32)
    for b in range(cfg.B):
        out[b] = res.results[2 * b]["y"] + res.results[2 * b + 1]["y"]
    return out, res


def kernel(x, prefix, W_qkv, W_out):
    x = np.asarray(x, np.float32)
    prefix = np.asarray(prefix, np.float32)
    W_qkv = np.asarray(W_qkv, np.float32)
    W_out = np.asarray(W_out, np.float32)
    out, _ = run(CFG, x, prefix, W_qkv, W_out)
    return out


# revision 52
# speedup vs baseline: 1.2173x; 1.1519x over previous
"""Trainium2 Bass kernel for prefix-causal self-attention (nn_CausalSelfAttention).

Reference semantics (B=4, T=2048, T_P=256, C=768, H=12, HD=64):
    x_full = concat([prefix, x], 1)                  (B, 2304, 768)
    qkv    = x_full @ W_qkv.T ; split q,k,v ; heads
    att    = softmax(mask(q k^T / sqrt(HD)))         prefix rows bidirectional,
                                                     x rows causal
    out    = (att v) heads-merged @ W_out.T ; return x-rows only (B, 2048, 768)

Sharding: 8 cores = 4 batches x 2 head-groups (tensor parallel on heads).
Each core computes Q/K/V for its 6 heads only (halving the K/V projection
work vs data-parallel-on-queries) over all 2304 kv rows and all 2048 query
rows, then the partial output projection y_g = O[:, g-heads] @ W_out^T[g].
The cross-group all-reduce of y is done on the HOST (numpy add of the two
partial results per batch) -- zero device cost. All 8 cores run an
identical instruction stream (true SPMD); only tensor data differs.

Query rows are processed in 16 chunks of 128 rows. With full-width rows per
chunk the causal-diagonal mask tile is the SAME lower-triangle [128x128] for
every chunk and head (one constant mask input), kv extents are exact
(et = 3+c tiles, no padding waste), and the S^T tile stream (6 heads x 168
tiles = 1008 tiles) packs perfectly into 126 PSUM quads of 8 tiles for exp.

On-chip pipeline per core (bf16 matmul operands, fp32 PSUM accumulation):
  QT/KT/V produced incrementally (one kv tile per chunk ahead of use) so
  attention starts early and PE gaps during exp are filled. Per S^T tile
  (kv 128 x q 128): S^T = K_h Q_h^T into a quad slot; one exp() per filled
  [128,1024] quad on ScalarE (softmax scale fused, no max-subtraction --
  scores are O(1) by construction); diagonal tiles get a triangle-mask
  multiply on DVE; AV emitted one quad behind the S stream with the
  orientation out[q,65] = pq^T V|1 (65-wide outputs halve the cost-model
  charge vs the [hd,q] orientation; the ones column gives the softmax
  denominator for free); per-head normalize via reciprocal + per-partition
  tensor_scalar broadcast (no DRAM bounce needed in this orientation);
  per-chunk O[q,384] is PE-transposed (identity matmul) to feed the W_out
  projection, y streamed out per chunk.
"""

import math
import os
from contextlib import ExitStack

import numpy as np
import ml_dtypes

import concourse.bass as bass
import concourse.bacc as bacc
import concourse.tile as tile
import concourse.mybir as mybir
from concourse._compat import with_exitstack

F32 = mybir.dt.float32
BF16 = mybir.dt.bfloat16
AF = mybir.ActivationFunctionType

# ---------------------------------------------------------------------------
# problem configuration
# ---------------------------------------------------------------------------


class Cfg:
    def __init__(self, B=4, T=2048, T_P=256, C=768, H=12):
        self.B, self.T, self.T_P, self.C, self.H = B, T, T_P, C, H
        self.HD = C // H
        assert self.HD == 64
        self.TALL = T_P + T
        assert self.TALL % 128 == 0 and T % 128 == 0 and T_P % 128 == 0
        self.NKV = self.TALL // 128          # kv tiles (18)
        self.CT = C // 128                   # contraction tiles over C (6)
        self.HG = H // 2                     # heads per core (6)
        self.CG = self.HG * self.HD          # feature cols per core (384)
        self.NP = self.CG // 128             # head pairs per core (3)
        self.NCH = T // 128                  # query chunks of 128 rows (16)
        self.PT = T_P // 128                 # prefix tiles (2)
        self.et = lambda c: self.PT + c + 1  # kv-tile extent of chunk c
        self.scale = 1.0 / math.sqrt(self.HD)


CFG = Cfg()

# ---------------------------------------------------------------------------
# device kernel (emitted once; same NEFF runs on all 8 cores)
# ---------------------------------------------------------------------------


@with_exitstack
def _emit(ctx: ExitStack, tc: tile.TileContext, cfg: Cfg, io: dict):
    nc = tc.nc
    C, CT, NP, NKV, NCH, CG = cfg.C, cfg.CT, cfg.NP, cfg.NKV, cfg.NCH, cfg.CG
    T = cfg.T

    xT_d, wq_d, wk_d, wv_d, wo_d, mk_d, y_d = (
        io["xT"], io["wqT"], io["wkT"], io["wvT"], io["woT"], io["mask"],
        io["y"])

    # ---- SBUF pools -------------------------------------------------------
    xT_p = ctx.enter_context(tc.tile_pool(name="xT", bufs=1))
    w_p = ctx.enter_context(tc.tile_pool(name="w", bufs=1))
    qT_p = ctx.enter_context(tc.tile_pool(name="qT", bufs=NP))
    kT_p = ctx.enter_context(tc.tile_pool(name="kT", bufs=NP))
    va_p = ctx.enter_context(tc.tile_pool(name="va", bufs=NKV))
    mk_p = ctx.enter_context(tc.tile_pool(name="mk", bufs=1))
    pq_p = ctx.enter_context(tc.tile_pool(name="pq", bufs=4))
    oc_p = ctx.enter_context(tc.tile_pool(name="oc", bufs=NCH))
    ot_p = ctx.enter_context(tc.tile_pool(name="ot", bufs=NCH * NP))
    nrm_p = ctx.enter_context(tc.tile_pool(name="nrm", bufs=8))
    y_p = ctx.enter_context(tc.tile_pool(name="ysb", bufs=NCH))
    # PSUM pools (8 banks): mm + quad(qx2 banks) + O
    MM_BUFS = int(os.environ.get("K_MM", "2"))
    QD_BUFS = int(os.environ.get("K_QD", "2"))
    O_BUFS = int(os.environ.get("K_O", "2"))
    assert MM_BUFS + 2 * QD_BUFS + O_BUFS <= 8
    mm_ps = ctx.enter_context(
        tc.tile_pool(name="mmps", bufs=MM_BUFS, space="PSUM"))
    qd_ps = ctx.enter_context(
        tc.tile_pool(name="qdps", bufs=QD_BUFS, space="PSUM"))
    o_ps = ctx.enter_context(
        tc.tile_pool(name="ops", bufs=O_BUFS, space="PSUM"))

    # ---- input loads ------------------------------------------------------
    # each logical tensor is ONE wide SBUF tile (per-ci column views) so it
    # can be filled by a single batched 3D-AP DMA
    wq_all = w_p.tile([128, CT * CG], BF16, tag="wq", name="wq_all")
    xT_all = xT_p.tile([128, CT * cfg.TALL], BF16, tag="xT", name="xT_all")
    wk_all = w_p.tile([128, CT * CG], BF16, tag="wk", name="wk_all")
    wv_all = w_p.tile([128, CT * CG], BF16, tag="wv", name="wv_all")
    wo_all = w_p.tile([128, NP * C], BF16, tag="wo", name="wo_all")
    wq = [wq_all[:, i * CG:(i + 1) * CG] for i in range(CT)]
    xT = [xT_all[:, i * cfg.TALL:(i + 1) * cfg.TALL] for i in range(CT)]
    wk = [wk_all[:, i * CG:(i + 1) * CG] for i in range(CT)]
    wv = [wv_all[:, i * CG:(i + 1) * CG] for i in range(CT)]
    wo = [wo_all[:, i * C:(i + 1) * C] for i in range(NP)]
    # xT arrives in column ranges so QT/KT/VA production can start early;
    # each (tensor, range) is ONE batched 3D-AP DMA -- the DGE pipeline is
    # serial per dma_start, so fewer/larger DMAs cut startup latency
    def dma_tiles(all_tile, src_ap, cols, lo=0, hi=None):
        hi = cols if hi is None else hi
        dst = all_tile[:].rearrange("p (c j) -> p c j", j=cols)[:, :, lo:hi]
        nc.sync.dma_start(
            dst, src_ap.rearrange("(c p) j -> p c j", p=128)[:, :, lo:hi])

    cuts = sorted({min(384, cfg.TALL), min(768, cfg.TALL),
                   min(1536, cfg.TALL), cfg.TALL})
    xr = [0] + [c for c in cuts if c > 0]
    dma_tiles(wk_all, wk_d, CG)
    dma_tiles(xT_all, xT_d, cfg.TALL, xr[0], xr[1])
    dma_tiles(wv_all, wv_d, CG)
    dma_tiles(wq_all, wq_d, CG)
    for j in range(1, len(xr) - 1):
        if j == 1:
            dma_tiles(xT_all, xT_d, cfg.TALL, xr[j], xr[j + 1])
            dma_tiles(wo_all, wo_d, C)
        else:
            dma_tiles(xT_all, xT_d, cfg.TALL, xr[j], xr[j + 1])
    # mask input: [128, 128] bf16 lower-triangle + [128, 128] f32 identity
    mk = mk_p.tile([128, 128], BF16, name="mk")
    nc.sync.dma_start(mk[:], mk_d[:])
    tri = mk[:, 0:128]
    ident = mk_p.tile([128, 128], BF16, name="ident")
    nc.sync.dma_start(ident[:], io["identf"][:])

    # HAM warmup: burn the input-DMA wait on dependency-free matmuls so the
    # PE p-state ramp (full speed only after ~3us of sustained activity) is
    # over before real work starts.
    warm = mk_p.tile([128, 512], BF16, name="warm")
    nc.vector.memset(warm[:], 1.0)
    for i in range(9):
        wps = mm_ps.tile([128, 512], F32, tag="mm", name=f"warmps{i}")
        nc.tensor.matmul(wps[:], warm[:, 0:128], warm[:],
                         start=True, stop=True)

    # ---- incremental producers -------------------------------------------
    QT = [qT_p.tile([128, T], BF16, tag="qT", name=f"QT{p}")
          for p in range(NP)]
    KT = [kT_p.tile([128, cfg.TALL], BF16, tag="kT", name=f"KT{p}")
          for p in range(NP)]
    VA = [va_p.tile([128, cfg.HG * 65], BF16, tag="va", name=f"VA{m}")
          for m in range(NKV)]

    def qt_block(p, n, w):
        ps = mm_ps.tile([128, w], F32, tag="mm", name=f"qps{p}_{n}")
        for ci in range(CT):
            nc.tensor.matmul(
                ps[:], wq[ci][:, bass.ts(p, 128)],
                xT[ci][:, cfg.T_P + n: cfg.T_P + n + w],
                start=(ci == 0), stop=(ci == CT - 1))
        nc.vector.tensor_copy(QT[p][:, n:n + w], ps[:])

    def kt_tile(p, t):
        ps = mm_ps.tile([128, 128], F32, tag="mm", name=f"kps{p}_{t}")
        for ci in range(CT):
            nc.tensor.matmul(
                ps[:], wk[ci][:, bass.ts(p, 128)],
                xT[ci][:, bass.ts(t, 128)],
                start=(ci == 0), stop=(ci == CT - 1))
        nc.vector.tensor_copy(KT[p][:, bass.ts(t, 128)], ps[:])

    def va_tile(m, part):
        hw = 3 * 64                        # 3 heads x 64 psum cols
        hv = 3 * 65                        # 3 heads x 65 VA cols
        vview = VA[m][:, part * hv:(part + 1) * hv].rearrange(
            "p (h c) -> p h c", c=65)
        nc.vector.memset(vview[:, :, 64:65], 1.0)
        ps = mm_ps.tile([128, hw], F32, tag="mm", name=f"vps{m}_{part}")
        for ci in range(CT):
            nc.tensor.matmul(
                ps[:], xT[ci][:, bass.ts(m, 128)],
                wv[ci][:, part * hw:(part + 1) * hw],
                start=(ci == 0), stop=(ci == CT - 1))
        nc.vector.tensor_copy(
            vview[:, :, 0:64], ps[:].rearrange("p (h c) -> p h c", c=64))

    # Work queue of PE filler items (KT/VA/QT production, deferred
    # projections): each item = (deadline_chunk_or_None, units, fn).
    # Deadline items MUST be emitted before that chunk's S matmuls enter the
    # PE queue (in-order engine); non-deadline items are spread evenly
    # across exp-paced quads to keep the PE fed.
    work = []
    wstat = {"emitted": 0, "quads": 0, "cycle": 0}
    # production pieces in deadline order (deadline = first chunk needing
    # the piece); pieces with deadline <= 0 are emitted before the stream.
    pieces = []
    for t in range(NKV):
        dl = t - cfg.PT
        for p in range(NP):
            pieces.append((dl, CT * 128, lambda p=p, t=t: kt_tile(p, t)))
        for part in range(2):
            pieces.append((dl, CT * 192,
                           lambda t=t, h=part: va_tile(t, h)))
    for n in range(0, T, 256):
        for p in range(NP):
            pieces.append((n // 128, CT * 256,
                           lambda p=p, n=n: qt_block(p, n, 256)))
    pieces.sort(key=lambda it: it[0])
    PRELOAD_DL = int(os.environ.get("K_PRE", "1"))
    for dl, u, fn in pieces:
        if dl <= PRELOAD_DL:
            fn()      # preload: early KT/VA tiles + QT cols
        else:
            work.append((dl, u, fn))

    # ---- attention chunk stream ------------------------------------------
    # Global pair-step stream packed into [128,1024] PSUM quads: each quad
    # holds 4 pair-steps; a pair-step emits the even head's S^T tile into a
    # bank-A slot (tile_position row 0) and the odd head's into the matching
    # bank-B slot (row 64). HW constraint (found the hard way): one PSUM
    # bank must not receive matmuls with different tile_position row bases
    # -- mixing 0/64 within a bank wedges the device. 504 pair-steps pack
    # exactly into 126 quads.
    total_steps = NP * sum(cfg.et(c) for c in range(NCH))
    state = {"qd": None, "pq": None, "cur": 0, "pending": [], "prev": [],
             "prev2": [], "pending3": [], "prev3": [], "prev3b": [],
             "masks": [], "step": 0}

    def flush_quad():
        """Close the current quad: exp + masks; drain previous quad's AV."""
        qd, pq = state["qd"], state["pq"]
        if qd is None:
            return
        if state["cur"] == 4:
            nc.scalar.activation(pq[:], qd[:], AF.Exp, scale=cfg.scale)
        else:
            w = 128 * state["cur"]
            qv = qd[:].rearrange("p (b s) -> p b s", s=512)[:, :, 0:w]
            pv = pq[:].rearrange("p (b s) -> p b s", s=512)[:, :, 0:w]
            nc.scalar.activation(pv, qv, AF.Exp, scale=cfg.scale)
        for emit_mask in state["masks"]:
            emit_mask()
        # drain the quad-before-last's AV/finish items: two quads of S
        # matmuls are queued ahead of each AV on PE, so exp has ~2 cycles
        # of lead time and the PE never parks on the Activation sem.
        for it in state["prev2"]:
            it()
        for it in state["prev3"]:
            it()
        state["prev3"] = state["prev3b"]
        state["prev3b"] = state["pending3"]
        state["pending3"] = []
        state["prev2"] = state["prev"]
        state["prev"] = state["pending"]
        state["pending"] = []
        state["qd"] = None
        state["masks"] = []
        wstat["quads"] += 1
        wstat["cycle"] = 0
        drain_work(budget=CYCLE_BUDGET)

    CYCLE_BUDGET = int(os.environ.get("K_BUDGET", "1400"))

    def drain_work(deadline=None, budget=0):
        # items: (dl, units, fn) or (dl, units, fn, min_quad). The budget
        # pass prefers deadline-carrying production items; deferred
        # projections/transposes are drawn only when no production remains,
        # so they accumulate as filler for the late (exp-paced) chunks.
        while True:
            if deadline is not None:
                idx = next((j for j, it in enumerate(work)
                            if it[0] is not None and it[0] <= deadline), None)
                if idx is not None:
                    item = work.pop(idx)
                    item[2]()
                    wstat["cycle"] += item[1]
                    wstat["emitted"] += item[1]
                    continue
            if not (budget > 0 and wstat["cycle"] < budget):
                break
            idx = next((j for j, it in enumerate(work)
                        if it[0] is not None), None)
            if idx is None:
                idx = next((j for j, it in enumerate(work)
                            if (it[3] if len(it) > 3 else 0)
                            <= wstat["quads"]), None)
            if idx is None:
                break
            item = work.pop(idx)
            item[2]()
            wstat["cycle"] += item[1]
            wstat["emitted"] += item[1]

    for c in range(NCH):
        et = cfg.et(c)
        qcols = bass.ts(c, 128)
        OC = oc_p.tile([128, CG], BF16, tag="oc", name=f"OC{c}")
        OTs = [ot_p.tile([128, 128], BF16, tag="ot", name=f"OT{c}_{p}")
               for p in range(NP)]
        for p in range(NP):
            O = o_ps.tile([128, 130], F32, tag="O", name=f"O{c}_{p}")
            for k in range(et):
                if state["qd"] is None:
                    state["qd"] = qd_ps.tile([128, 1024], F32, tag="qd",
                                             name="qd")
                    state["pq"] = pq_p.tile([128, 1024], BF16, tag="pq",
                                            name="pq")
                    state["cur"] = 0
                cur = state["cur"]
                qd, pq = state["qd"], state["pq"]
                for ho in range(2):
                    h, hp, s = 2 * p + ho, 64 * ho, cur + 4 * ho
                    nc.tensor.matmul(
                        qd[:, bass.ts(s, 128)],
                        KT[p][hp:hp + 64, bass.ts(k, 128)],
                        QT[p][hp:hp + 64, qcols],
                        start=(cur == 0),
                        stop=(cur == 3 or state["step"] == total_steps - 1))
                    if k == et - 1:
                        eng = (nc.gpsimd if os.environ.get(
                            "K_MASK", "dve") == "pool" else nc.vector)
                        state["masks"].append(
                            lambda pq=pq, s=s, eng=eng:
                            eng.tensor_mul(pq[:, bass.ts(s, 128)],
                                           pq[:, bass.ts(s, 128)], tri))

                def av_item(pq=pq, cur=cur, O=O, p=p, k=k, et=et):
                    for ho in range(2):
                        nc.tensor.matmul(
                            O[:, 65 * ho:65 * ho + 65],
                            pq[:, bass.ts(cur + 4 * ho, 128)],
                            VA[k][:, 65 * (2 * p + ho):65 * (2 * p + ho) + 65],
                            start=(k == 0 and ho == 0),
                            stop=(k == et - 1 and ho == 1))
                if k == et - 1 and et >= int(os.environ.get("K_D3", "99")):
                    # diagonal tiles wait on the Pool mask multiply; give
                    # them (and the pair finish) one extra cycle of lead.
                    # Only for big chunks: early chunks cycle O banks too
                    # fast for the extra deferral.
                    state["pending3"].append(av_item)
                else:
                    state["pending"].append(av_item)
                state["cur"] += 1
                state["step"] += 1
                if state["cur"] == 4:
                    flush_quad()

            def fin_pair(O=O, p=p, c=c, OC=OC):
                recip = nrm_p.tile([128, 2], F32, tag="recip",
                                   name=f"rc{c}_{p}")
                dview = O[:].rearrange("q (h c) -> q h c", c=65)
                nc.vector.reciprocal(recip[:], dview[:, :, 64])
                for ho in range(2):
                    nc.vector.tensor_scalar_mul(
                        OC[:, 128 * p + 64 * ho:128 * p + 64 * ho + 64],
                        O[:, 65 * ho:65 * ho + 64], recip[:, ho:ho + 1])
            (state["pending3"] if et >= int(os.environ.get("K_D3", "99"))
             else state["pending"]).append(fin_pair)

        def transp(c=c, OC=OC, OTs=OTs):
            for p in range(NP):
                tp = mm_ps.tile([128, 128], BF16, tag="mm", name=f"tp{c}_{p}")
                nc.tensor.transpose(tp[:], OC[:, bass.ts(p, 128)], ident)
                nc.vector.tensor_copy(OTs[p][:], tp[:])

        def mk_proj(c, OTs, n, w, ysb):
            def proj():
                ps = mm_ps.tile([128, w], F32, tag="mm", name=f"yps{c}_{n}")
                for p in range(NP):
                    nc.tensor.matmul(ps[:], OTs[p][:], wo[p][:, n:n + w],
                                     start=(p == 0), stop=(p == NP - 1))
                nc.vector.tensor_copy(ysb[:, n:n + w], ps[:])
                if n + w == C:
                    nc.sync.dma_start(y_d[bass.ts(c, 128), :], ysb[:])
            return proj

        def fin_chunk(c=c, OC=OC, OTs=OTs):
            # deferred PE filler: transposes + output projection; emitted
            # later by the budget drain so late (exp-paced) quads stay fed
            ysb = y_p.tile([128, C], F32, tag="ysb", name=f"ysb{c}")
            g0 = int(os.environ.get("K_GATE", "2"))
            gate = wstat["quads"] + g0
            work.append((None, NP * 128, lambda: transp(c, OC, OTs), gate))
            for n in range(0, C, 256):
                work.append((None, NP * 256, mk_proj(c, OTs, n, 256, ysb),
                             gate + 1))
        state["pending"].append(fin_chunk)
        drain_work(deadline=c + 2)

    flush_quad()  # close the final quad (no-op when it ended exactly full)
    for it in state["prev2"]:
        it()
    for it in state["prev3"]:
        it()
    for it in state["prev3b"]:
        it()
    for it in state["prev"]:
        it()
    for it in state["pending"]:
        it()
    for it in state["pending3"]:
        it()
    state["pending"] = []
    state["pending3"] = []
    while work:
        item = work.pop(0)
        item[2]()
    assert state["qd"] is None


def build_nc(cfg: Cfg):
    nc = bacc.Bacc("TRN2", target_bir_lowering=False, debug=False,
                   enable_asserts=False)
    io = {
        "xT": nc.dram_tensor("xT", (cfg.C, cfg.TALL), BF16,
                             kind="ExternalInput").ap(),
        "wqT": nc.dram_tensor("wqT", (cfg.C, cfg.CG), BF16,
                              kind="ExternalInput").ap(),
        "wkT": nc.dram_tensor("wkT", (cfg.C, cfg.CG), BF16,
                              kind="ExternalInput").ap(),
        "wvT": nc.dram_tensor("wvT", (cfg.C, cfg.CG), BF16,
                              kind="ExternalInput").ap(),
        "woT": nc.dram_tensor("woT", (cfg.CG, cfg.C), BF16,
                              kind="ExternalInput").ap(),
        "mask": nc.dram_tensor("mask", (128, 128), BF16,
                               kind="ExternalInput").ap(),
        "identf": nc.dram_tensor("identf", (128, 128), BF16,
                                 kind="ExternalInput").ap(),
        "y": nc.dram_tensor("y", (cfg.T, cfg.C), F32,
                            kind="ExternalOutput").ap(),
    }
    with tile.TileContext(nc) as tc:
        _emit(tc, cfg, io)
    nc.compile()
    return nc


# ---------------------------------------------------------------------------
# host side: shard, run, gather
# ---------------------------------------------------------------------------


def _in_maps(cfg: Cfg, x, prefix, W_qkv, W_out):
    C, CG = cfg.C, cfg.CG
    mask = np.triu(np.ones((128, 128), np.float32)  # mask[i,j]=1 iff j>=i
                   ).astype(ml_dtypes.bfloat16)
    identf = np.eye(128, dtype=np.float32).astype(ml_dtypes.bfloat16)
    xTs = []
    for b in range(cfg.B):
        xT = np.ascontiguousarray(
            np.concatenate([prefix[b], x[b]], axis=0).T
        ).astype(ml_dtypes.bfloat16)
        xTs.append(xT)
    maps = []
    for core in range(2 * cfg.B):
        b, g = divmod(core, 2)
        sl = slice(CG * g, CG * (g + 1))
        maps.append({
            "xT": xTs[b],
            "wqT": np.ascontiguousarray(W_qkv[0:C][sl].T
                                        ).astype(ml_dtypes.bfloat16),
            "wkT": np.ascontiguousarray(W_qkv[C:2 * C][sl].T
                                        ).astype(ml_dtypes.bfloat16),
            "wvT": np.ascontiguousarray(W_qkv[2 * C:][sl].T
                                        ).astype(ml_dtypes.bfloat16),
            "woT": np.ascontiguousarray(W_out[:, sl].T
                                        ).astype(ml_dtypes.bfloat16),
            "mask": mask,
            "identf": identf,
        })
    return maps


_NC_CACHE = {}


def run(cfg: Cfg, x, prefix, W_qkv, W_out, **kw):
    from concourse.bass_utils import run_bass_kernel_spmd
    key = (cfg.B, cfg.T, cfg.T_P, cfg.C, cfg.H)
    if key not in _NC_CACHE:
        _NC_CACHE[key] = build_nc(cfg)
    nc = _NC_CACHE[key]
    maps = _in_maps(cfg, x, prefix, W_qkv, W_out)
    res = run_bass_kernel_spmd(nc, maps, core_ids=list(range(2 * cfg.B)), **kw)
    out = np.empty((cfg.B, cfg.T, cfg.C), np.float32)
    for b in range(cfg.B):
        out[b] = res.results[2 * b]["y"] + res.results[2 * b + 1]["y"]
    return out, res


def kernel(x, prefix, W_qkv, W_out):
    x = np.asarray(x, np.float32)
    prefix = np.asarray(prefix, np.float32)
    W_qkv = np.asarray(W_qkv, np.float32)
    W_out = np.asarray(W_out, np.float32)
    out, _ = run(CFG, x, prefix, W_qkv, W_out)
    return out
